# revision 52
# baseline (speedup 1.0000x reference)
"""Trainium2 Bass kernel for nn_DecodeMoeOps (MoE decode: dispatch-quant,
grouped int8 GEMM1, SwiGLU, requant, grouped int8 GEMM2, weighted combine).

Expert-parallel across 8 NeuronCores: core c owns experts {2c, 2c+1}. Each
core computes only the tokens routed to its experts (gathered host-side,
padded to N_PAD), using weight-stationary GEMMs over routed tokens:

  GEMM1: out[f, tok] = w1_tile[k,f].T @ xqs[k, tok]   (xqs = fp16(xq*sx))
  GEMM2: out[h, tok] = w2_tile[i,h].T @ aq[i, tok]

w1 ships as int8 and is cast to fp16 on-chip (split across DVE/ACT/GPSIMD);
w2 ships as fp8e3m4 (exact for |w|<=31, max abs err 2 above) with the 16x
scale folded into w2_scale. Per-channel dequant scales are per-partition in
this layout; the requant absmax runs on GPSIMD partition_all_reduce. Host
scatters the per-expert [h, tok] outputs back into y[B, H].
"""

import os
import sys

for _p in ("/opt/trn_rl_repo", "/root/.axon_site/_ro/trn_rl_repo"):
    if os.path.isdir(_p) and _p not in sys.path:
        sys.path.insert(0, _p)

from contextlib import ExitStack

import ml_dtypes
import numpy as np

import concourse.bass as bass
import concourse.mybir as mybir
import concourse.tile as tile
from concourse import bacc
from concourse import bass_isa
from concourse.bass_utils import run_bass_kernel_spmd

B, TOPK, H, I, E = 128, 8, 2048, 1408, 16
NCORES = 8
EPC = E // NCORES  # experts per core
KH = H // 128  # 16 k-tiles for GEMM1 contraction
KI = I // 128  # 11 k-tiles for GEMM2 contraction
FT = I // 128  # 11 f-tiles per GEMM1 half
HT = H // 128  # 16 h-tiles for GEMM2 output
I2 = 2 * I
F32 = mybir.dt.float32
BF16 = mybir.dt.bfloat16
F16 = mybir.dt.float16
I8 = mybir.dt.int8
F8E3 = mybir.dt.float8e3
MAGIC = float(3 * 2**22)  # fp32 round-to-int magic (covers negatives)

# on-chip int8->fp16 cast: free-dim split of each [128, 2816] w1 k-tile
CAST_DVE = (0, 1536)
CAST_ACT = (1536, 704)
CAST_POOL = (2240, 576)

_cache: dict = {}


def _build_program(n_pad: int):
    mult = mybir.AluOpType.mult
    nc = bacc.Bacc(
        "TRN2",
        target_bir_lowering=False,
        debug=False,
        num_devices=NCORES,
    )

    FW = FT * n_pad   # gate/up accumulator width
    HW = HT * n_pad   # GEMM2 accumulator width

    # --- per-core DRAM I/O ---
    xqsT_d = nc.dram_tensor("xqsT", [EPC, 128, KH, n_pad], F16, kind="ExternalInput").ap()
    w1_d = nc.dram_tensor("w1t", [EPC, KH, 128, I2], I8, kind="ExternalInput").ap()
    w2_d = nc.dram_tensor("w2t", [EPC, KI, 128, H], F8E3, kind="ExternalInput").ap()
    s1g_d = nc.dram_tensor("s1g", [EPC, 128, FT], F32, kind="ExternalInput").ap()
    s1gu_d = nc.dram_tensor("s1gu", [EPC, 128, FT], F32, kind="ExternalInput").ap()
    w2s_d = nc.dram_tensor("scale2", [EPC, 128, HT], F32, kind="ExternalInput").ap()
    comb_d = nc.dram_tensor("combs", [EPC, 128, n_pad], F32, kind="ExternalInput").ap()
    o_d = nc.dram_tensor("o", [EPC, 128, HW], BF16, kind="ExternalOutput").ap()

    with tile.TileContext(nc) as tc, ExitStack() as ctx:
        consts = ctx.enter_context(tc.tile_pool(name="consts", bufs=1))
        w1i8p = ctx.enter_context(tc.tile_pool(name="w1i8", bufs=5))
        w1f16p = ctx.enter_context(tc.tile_pool(name="w1f16", bufs=5))
        w2p = ctx.enter_context(tc.tile_pool(name="w2p", bufs=1))
        epi = ctx.enter_context(tc.tile_pool(name="epi", bufs=2))
        op_ = ctx.enter_context(tc.tile_pool(name="op", bufs=2))
        ps1_pool = ctx.enter_context(tc.tile_pool(name="ps1", bufs=2, space="PSUM"))
        ps2_pool = ctx.enter_context(tc.tile_pool(name="ps2", bufs=1, space="PSUM"))

        # --- prologue: small inputs ---
        xqs_s = consts.tile([128, EPC, KH, n_pad], F16, name="xqs_s")
        nc.scalar.dma_start(out=xqs_s[:], in_=xqsT_d.rearrange("e p k j -> p e k j"))
        s1g_s = consts.tile([128, EPC, FT], F32, name="s1g_s")
        nc.scalar.dma_start(out=s1g_s[:], in_=s1g_d.rearrange("e p t -> p e t"))
        s1gu_s = consts.tile([128, EPC, FT], F32, name="s1gu_s")
        nc.scalar.dma_start(out=s1gu_s[:], in_=s1gu_d.rearrange("e p t -> p e t"))
        w2s_s = consts.tile([128, EPC, HT], F32, name="w2s_s")
        nc.scalar.dma_start(out=w2s_s[:], in_=w2s_d.rearrange("e p t -> p e t"))
        comb_s = consts.tile([128, EPC, n_pad], F32, name="comb_s")
        nc.scalar.dma_start(out=comb_s[:], in_=comb_d.rearrange("e p j -> p e j"))

        def bank_flags(offsets_bytes):
            """PSUM accumulation start/stop flags per chunk: matmul start=True
            zeroes the whole 2KB bank, so exactly one start (first chunk) and
            one stop (last chunk) per bank. Offsets must not cross banks."""
            first, last = {}, {}
            for i, off in enumerate(offsets_bytes):
                b = off // 2048
                if b not in first:
                    first[b] = i
                last[b] = i
            starts = {i for i in first.values()}
            stops = {i for i in last.values()}
            return starts, stops

        def epilogue1_ops(e, ps1_e, out, last=False):
            """dequant + SwiGLU + requant -> aq; returns one closure per op
            so the caller can interleave emission with other work. For the
            final expert (last=True) the requant is chunked per GEMM2 k-tile
            and kept off GPSIMD so the tail chain is as short as possible."""
            ps_g = ps1_e[:, 0:FW]
            ps_u = ps1_e[:, FW : 2 * FW]
            s1g_b = epi.tile([128, FT, n_pad], F32, tag="s1gb", name=f"s1gb_{e}")
            s1gu_b = epi.tile([128, FT, n_pad], F32, tag="s1gub", name=f"s1gub_{e}")
            gate = epi.tile([128, FW], F32, tag="gate", name=f"gate_{e}")
            sig = epi.tile([128, FW], F32, tag="sig", name=f"sig_{e}")
            gdu = epi.tile([128, FW], F32, tag="gdu", name=f"gdu_{e}")
            t1 = epi.tile([128, FW], F32, tag="t1", name=f"t1_{e}")
            act2 = epi.tile([128, FW], F32, tag="act2", name=f"act2_{e}")
            am = epi.tile([128, FW], F32, tag="am", name=f"am_{e}")
            m = epi.tile([128, n_pad], F32, tag="m", name=f"m_{e}")
            mc = epi.tile([128, n_pad], F32, tag="mc", name=f"mc_{e}")
            r = epi.tile([128, n_pad], F32, tag="r", name=f"r_{e}")
            tq = epi.tile([128, FW], F32, tag="tq", name=f"tq_{e}")
            tq2 = epi.tile([128, FW], F32, tag="tq2", name=f"tq2_{e}")
            aq = epi.tile([128, FT, n_pad], BF16, tag="aq", name=f"aq_{e}")
            s2c = epi.tile([128, n_pad], F32, tag="s2c", name=f"s2c_{e}")
            w2sc = epi.tile([128, HT, n_pad], F32, tag="w2sc", name=f"w2sc_{e}")
            out["aq"], out["w2sc"] = aq, w2sc
            ops = [
                lambda: nc.vector.tensor_copy(
                    out=s1g_b[:],
                    in_=s1g_s[:, e, :].unsqueeze(2).broadcast_to([128, FT, n_pad])),
                lambda: nc.vector.tensor_copy(
                    out=s1gu_b[:],
                    in_=s1gu_s[:, e, :].unsqueeze(2).broadcast_to([128, FT, n_pad])),
                lambda: nc.vector.tensor_tensor(
                    out=gate[:], in0=ps_g,
                    in1=s1g_b[:].rearrange("p t n -> p (t n)"), op=mult),
                lambda: nc.scalar.activation(
                    out=sig[:], in_=gate[:],
                    func=mybir.ActivationFunctionType.Sigmoid),
                lambda: nc.vector.tensor_tensor(
                    out=gdu[:], in0=ps_g,
                    in1=s1gu_b[:].rearrange("p t n -> p (t n)"), op=mult),
                (lambda: nc.vector.tensor_tensor(
                    out=t1[:], in0=gdu[:], in1=sig[:], op=mult)) if last else
                (lambda: nc.gpsimd.tensor_tensor(
                    out=t1[:], in0=gdu[:], in1=sig[:], op=mult)),
                lambda: nc.vector.tensor_tensor(
                    out=act2[:], in0=t1[:], in1=ps_u, op=mult),
                lambda: nc.gpsimd.partition_all_reduce(
                    am[:], act2[:], channels=128,
                    reduce_op=bass_isa.ReduceOp.absmax),
                lambda: nc.vector.tensor_reduce(
                    out=m[:], in_=am[:].rearrange("p (t n) -> p n t", t=FT),
                    op=mybir.AluOpType.max, axis=mybir.AxisListType.X),
                lambda: nc.vector.tensor_scalar_max(
                    out=mc[:], in0=m[:], scalar1=1e-12),
                lambda: nc.vector.reciprocal(out=r[:], in_=mc[:]),
            ]
            tqv = tq[:].rearrange("p (t n) -> p t n", t=FT)
            tq2v = tq2[:].rearrange("p (t n) -> p t n", t=FT)
            a2v = act2[:].rearrange("p (t n) -> p t n", t=FT)
            if not last:
                ops += [
                    lambda: nc.vector.scalar_tensor_tensor(
                        out=tqv, in0=a2v, scalar=127.0,
                        in1=r[:].unsqueeze(1).broadcast_to([128, FT, n_pad]),
                        op0=mult, op1=mult),
                    lambda: nc.scalar.activation(
                        out=tq2[:], in_=tq[:],
                        func=mybir.ActivationFunctionType.Copy, bias=MAGIC),
                    lambda: nc.scalar.activation(
                        out=aq[:].rearrange("p t n -> p (t n)"), in_=tq2[:],
                        func=mybir.ActivationFunctionType.Copy, bias=-MAGIC),
                ]
            else:
                # per-GEMM2-k-tile requant: aq chunk ki is ready ~3 small ops
                # after r, so GEMM2 can start immediately
                for ki in range(KI):
                    ops += [
                        (lambda ki=ki: nc.vector.scalar_tensor_tensor(
                            out=tqv[:, ki : ki + 1, :],
                            in0=a2v[:, ki : ki + 1, :], scalar=127.0,
                            in1=r[:].unsqueeze(1).broadcast_to([128, 1, n_pad]),
                            op0=mult, op1=mult)),
                        (lambda ki=ki: nc.scalar.activation(
                            out=tq2v[:, ki : ki + 1, :], in_=tqv[:, ki : ki + 1, :],
                            func=mybir.ActivationFunctionType.Copy, bias=MAGIC)),
                        (lambda ki=ki: nc.vector.tensor_scalar_add(
                            out=aq[:, ki : ki + 1, :], in0=tq2v[:, ki : ki + 1, :],
                            scalar1=-MAGIC)),
                    ]
            ops += [
                lambda: nc.vector.scalar_tensor_tensor(
                    out=s2c[:], in0=mc[:], scalar=1.0 / 127.0,
                    in1=comb_s[:, e, :], op0=mult, op1=mult),
                lambda: nc.gpsimd.tensor_tensor(
                    out=w2sc[:],
                    in0=w2s_s[:, e, :].unsqueeze(2).broadcast_to([128, HT, n_pad]),
                    in1=s2c[:].unsqueeze(1).broadcast_to([128, HT, n_pad]),
                    op=mult),
            ]
            return ops

        def gemm2(e, aq, w2sc, w2tiles):
            """weight-stationary GEMM2 + per-bank dequant + output DMA.
            PSUM can only be read once a bank's accumulation group stopped,
            so the last-ki MMs and the dequant proceed bank by bank."""
            ps2 = ps2_pool.tile([128, HW], F32, tag="ps2", name=f"ps2_{e}")
            o_sb = op_.tile([128, HW], BF16, tag="o_sb", name=f"o_{e}")
            cpb = max(1, 2048 // (n_pad * 4))  # h-chunks per PSUM bank
            banks = [list(range(b, min(b + cpb, HT))) for b in range(0, HT, cpb)]
            final_ki = KI - 1
            dve_tail = e == EPC - 1
            for ki in range(KI):
                last = ki == final_ki
                for bi, bchunks in enumerate(banks):
                    for t in bchunks:
                        if last and dve_tail:
                            ha_t, hb_t, tsplit = w2half
                            if t < tsplit:
                                lhsT = ha_t[:, t * 128 : (t + 1) * 128]
                            else:
                                lhsT = hb_t[:, (t - tsplit) * 128 : (t - tsplit + 1) * 128]
                        else:
                            lhsT = w2tiles[ki][:, t * 128 : (t + 1) * 128]
                        nc.tensor.matmul(
                            ps2[:, t * n_pad : (t + 1) * n_pad],
                            lhsT=lhsT,
                            rhs=aq[:, ki, :],
                            start=(ki == 0 and t in g2_starts),
                            stop=(last and t in g2_stops),
                        )
                    if not last:
                        continue
                    if dve_tail:
                        if bi == len(banks) - 1:
                            # single full-width dequant + one output DMA:
                            # both banks have stopped, and one op avoids the
                            # inter-op semaphore slop on the tail
                            nc.vector.tensor_tensor(
                                out=o_sb[:],
                                in0=ps2[:],
                                in1=w2sc[:].rearrange("p t n -> p (t n)"),
                                op=mult,
                            )
                            nc.sync.dma_start(out=o_d[e], in_=o_sb[:])
                        continue
                    lo = bchunks[0] * n_pad
                    hi = (bchunks[-1] + 1) * n_pad
                    if bi < len(banks) - 1 and not dve_tail:
                        # earlier banks: ACT copies psum out, GPSIMD scales
                        od = epi.tile([128, hi - lo], F32, tag="odeq", name=f"od_{e}_{bi}")
                        nc.scalar.activation(
                            out=od[:], in_=ps2[:, lo:hi],
                            func=mybir.ActivationFunctionType.Copy,
                        )
                        nc.gpsimd.tensor_tensor(
                            out=o_sb[:, lo:hi],
                            in0=od[:],
                            in1=w2sc[:, bchunks[0] : bchunks[-1] + 1, :].rearrange(
                                "p t n -> p (t n)"
                            ),
                            op=mult,
                        )
                    else:
                        # final bank: one DVE op for the shortest tail chain
                        nc.vector.tensor_tensor(
                            out=o_sb[:, lo:hi],
                            in0=ps2[:, lo:hi],
                            in1=w2sc[:, bchunks[0] : bchunks[-1] + 1, :].rearrange(
                                "p t n -> p (t n)"
                            ),
                            op=mult,
                        )
                    nc.sync.dma_start(
                        out=o_d[e, :, lo:hi], in_=o_sb[:, lo:hi]
                    )

        # --- main pipeline: expert-sequential so epilogue(e0) hides under
        # --- expert 1's cast stream; w2(e1) is the last DMA (tail-paced)
        g1_chunks = [(h, t) for h in (0, 1) for t in range(FT)]
        g1_chunks_last = [(h, t) for h in (1, 0) for t in range(FT)]
        s_idx, _ = bank_flags([h * FW * 4 + t * n_pad * 4 for h, t in g1_chunks])
        _, e_idx = bank_flags([h * FW * 4 + t * n_pad * 4 for h, t in g1_chunks_last])
        g1_starts = {g1_chunks[i] for i in s_idx}
        g1_stops = {g1_chunks_last[i] for i in e_idx}
        g2_starts, g2_stops = bank_flags([t * n_pad * 4 for t in range(HT)])
        w2tiles = {}
        epi_res = {e: {} for e in range(EPC)}
        pending = []
        for e in range(EPC):
            ps1_e = ps1_pool.tile([128, 2 * FW], F32, tag="ps1", name=f"ps1_{e}")
            for k in range(KH):
                w1i8 = w1i8p.tile([128, I2], I8, tag="w1i8", name=f"w1i8_{e}_{k}")
                nc.sync.dma_start(out=w1i8[:], in_=w1_d[e, k])
                w1f = w1f16p.tile([128, I2], F16, tag="w1f", name=f"w1f_{e}_{k}")
                o0, n0 = CAST_DVE
                nc.vector.tensor_copy(out=w1f[:, o0 : o0 + n0], in_=w1i8[:, o0 : o0 + n0])
                o1, n1 = CAST_ACT
                nc.scalar.activation(
                    out=w1f[:, o1 : o1 + n1],
                    in_=w1i8[:, o1 : o1 + n1],
                    func=mybir.ActivationFunctionType.Copy,
                )
                o2, n2 = CAST_POOL
                nc.gpsimd.tensor_copy(out=w1f[:, o2 : o2 + n2], in_=w1i8[:, o2 : o2 + n2])
                rhs = xqs_s[:, e, k, :]
                chunks = g1_chunks if k < KH - 1 else g1_chunks_last
                for half, t in chunks:
                    base = half * FW
                    nc.tensor.matmul(
                        ps1_e[:, base + t * n_pad : base + (t + 1) * n_pad],
                        lhsT=w1f[:, half * I + t * 128 : half * I + (t + 1) * 128],
                        rhs=rhs,
                        start=(k == 0 and (half, t) in g1_starts),
                        stop=(k == KH - 1 and (half, t) in g1_stops),
                    )
                # sprinkle the previous expert's epilogue into this cast
                # stream so its cross-engine chain never stalls the casts
                if pending and k >= 1:
                    pending.pop(0)()
            while pending:
                pending.pop(0)()
            pending = epilogue1_ops(e, ps1_e, epi_res[e], last=(e == EPC - 1))
        # all w2 DMAs ride behind the full w1 stream in the SP queue; the
        # very last k-tile (last expert) ships as two half-width tiles
        w2half = {}
        for e in range(EPC):
            w2tiles[e] = []
            nk = KI - 1 if e == EPC - 1 else KI
            for ki in range(nk):
                w2t = w2p.tile([128, H], F8E3, tag=f"w2_{e}_{ki}", name=f"w2_{e}_{ki}")
                nc.sync.dma_start(out=w2t[:], in_=w2_d[e, ki])
                w2tiles[e].append(w2t)
        e = EPC - 1
        # asymmetric split: the final transfer is only 4 h-chunks so the very
        # last bytes gate just 4 matmuls
        HSPLIT = 12 * 128
        ha = w2p.tile([128, HSPLIT], F8E3, tag="w2ha", name="w2_last_a")
        nc.sync.dma_start(out=ha[:], in_=w2_d[e, KI - 1][:, 0:HSPLIT])
        hb = w2p.tile([128, H - HSPLIT], F8E3, tag="w2hb", name="w2_last_b")
        nc.sync.dma_start(out=hb[:], in_=w2_d[e, KI - 1][:, HSPLIT:H])
        w2half = (ha, hb, HSPLIT // 128)
        # last expert's epilogue emitted inline
        while pending:
            pending.pop(0)()
        for e in range(EPC):
            gemm2(e, epi_res[e]["aq"], epi_res[e]["w2sc"], w2tiles[e])

    nc.compile()
    return nc


def get_program(n_pad: int):
    key = ("nc", n_pad)
    if key not in _cache:
        _cache[key] = _build_program(n_pad)
    return _cache[key]


def _routing(expert_ids, expert_scales):
    """comb[B, E] scatter-add; token lists per expert; N_PAD."""
    comb = np.zeros((B, E), np.float32)
    np.add.at(comb, (np.arange(B)[:, None], np.asarray(expert_ids)),
              np.asarray(expert_scales, np.float32))
    routed = np.zeros((B, E), bool)
    routed[np.arange(B)[:, None], np.asarray(expert_ids)] = True
    toks = [np.nonzero(routed[:, e])[0] for e in range(E)]
    max_n = max(len(t) for t in toks)
    n_pad = 16
    while n_pad < max_n:
        n_pad *= 2
    # PSUM chunking requires pow2 n_pad; >64 would overflow the 8 banks
    assert n_pad <= 64, f"routing too dense for this kernel: n_pad={n_pad}"
    return comb, toks, n_pad


def _prep_inputs(x, expert_ids, smooth_scales, expert_scales, w1, w1_scale, w2, w2_scale):
    """Host-side dispatch: quantize x, route tokens, shard experts."""
    x = np.asarray(x, np.float32)
    smooth_scales = np.asarray(smooth_scales, np.float32)
    w1_scale = np.asarray(w1_scale, np.float32)
    w2_scale = np.asarray(w2_scale, np.float32)

    # dynamic per-token int8 quantization (exact mirror of reference ops)
    sx = np.maximum(np.max(np.abs(x), axis=-1, keepdims=True), 1e-12) / 127.0
    xq = np.round(np.clip(x / sx, -128.0, 127.0)).astype(np.float32)
    xqs = (xq * sx).astype(np.float16)  # [B, H]
    xqsT = np.ascontiguousarray(
        xqs.T.reshape(KH, 128, B).transpose(1, 0, 2)
    )  # [128, KH, B]

    comb, toks, n_pad = _routing(expert_ids, expert_scales)

    w1v = np.asarray(w1).astype(np.int8)
    w2v = np.asarray(w2).astype(np.int8)

    in_maps = []
    for c in range(NCORES):
        es = list(range(c * EPC, (c + 1) * EPC))
        xqsT_e = np.zeros((EPC, 128, KH, n_pad), np.float16)
        comb_e = np.zeros((EPC, 128, n_pad), np.float32)
        for i, e in enumerate(es):
            tk = toks[e]
            xqsT_e[i, :, :, : len(tk)] = xqsT[:, :, tk]
            comb_e[i, :, : len(tk)] = comb[tk, e][None, :]
        w1c = w1v[es].reshape(EPC, KH, 128, I2)
        w2c = np.ascontiguousarray(
            (w2v[es].reshape(EPC, KI, 128, H).astype(np.float32) / 16.0)
        ).astype(ml_dtypes.float8_e3m4)
        # per-partition scale columns [e, p, T]
        s1g_full = w1_scale[es][:, :I]
        s1u_full = w1_scale[es][:, I:] * smooth_scales[es]
        s1g = np.ascontiguousarray(s1g_full.reshape(EPC, FT, 128).transpose(0, 2, 1))
        s1gu = np.ascontiguousarray(
            (s1g_full * s1u_full).reshape(EPC, FT, 128).transpose(0, 2, 1))
        sc2 = np.ascontiguousarray(
            (w2_scale[es] * 16.0).reshape(EPC, HT, 128).transpose(0, 2, 1))
        in_maps.append(
            {
                "xqsT": xqsT_e,
                "w1t": np.ascontiguousarray(w1c),
                "w2t": w2c,
                "s1g": s1g.astype(np.float32),
                "s1gu": s1gu.astype(np.float32),
                "scale2": sc2.astype(np.float32),
                "combs": comb_e,
            }
        )
    return in_maps, toks, n_pad


def kernel(
    x,
    expert_ids,
    smooth_scales,
    expert_scales,
    x_active_mask,
    w1,
    w1_scale,
    w2,
    w2_scale,
    _trace=False,
    _trace_kwargs=None,
):
    in_maps, toks, n_pad = _prep_inputs(
        x, expert_ids, smooth_scales, expert_scales, w1, w1_scale, w2, w2_scale
    )
    nc = get_program(n_pad)
    res = run_bass_kernel_spmd(
        nc,
        in_maps,
        core_ids=list(range(NCORES)),
        trace=_trace,
        **(_trace_kwargs or {}),
    )
    y = np.zeros((B, H), np.float32)
    for c, r in enumerate(res.results):
        o = np.asarray(r["o"], np.float32).reshape(EPC, 128, HT, n_pad)
        for i in range(EPC):
            e = c * EPC + i
            tk = toks[e]
            contrib = o[i, :, :, : len(tk)].transpose(2, 1, 0).reshape(len(tk), H)
            y[tk] += contrib
    y *= np.asarray(x_active_mask).astype(np.float32)[:, None]
    if _trace:
        kernel.last_results = res
    return y
